# revision 1
# baseline (speedup 1.0000x reference)
"""DelayAttention Trainium2 kernel.

Data-parallel over batch: B=16 split as 2 batches per core across 8 cores.
All params replicated. Per core, per batch, the sequence is processed in
512-token slices (tokens = (t, n) pairs, 64 nodes per timestep):

  1. DMA x slice, transpose on PE -> xT [d, tok] (fp32 exact).
  2. Linears Q/K/V/u as f32r matmuls (weights stationary, K-chunked over d),
     outputs live transposed [dk, tok] which is what attention wants.
  3. sim[p, tok] = sum_{s,d} m[p,s,d] * u[tok + 64 s, d] via 10 accumulated
     matmuls (M=8) against a sliding window of the persistent UT buffer.
  4. pattern softmax: exp (no max needed, |sim| < ~40), denominator via
     ones-matmul (partition reduction on PE), reciprocal, w = e * rd.
  5. r = c_sum.T @ w accumulated directly into the K-linear PSUM group.
  6. attention in bf16: per pair of timesteps, scores packed into one
     [128, 64] psum via column-tiling; softmax over free dim; attn
     transposed on PE; attn@V with unnormalized weights; normalization by
     1/sum(exp) folded into the per-partition output scale.

Host-side prep (tiny, O(params)): weight transposes, m = patterns@Wm.T+bm
reshaped to [dk, (s,p)], c_sum = (patterns@Wc.T+bc).sum(s), identities.
"""

import os
import sys

import numpy as np

for _p in ("/opt/trn_rl_repo",):
    if _p not in sys.path and os.path.isdir(_p):
        sys.path.insert(0, _p)

import ml_dtypes  # noqa: E402

import concourse.bass as bass  # noqa: E402
import concourse.mybir as mybir  # noqa: E402
import concourse.tile as tile  # noqa: E402
from concourse import bacc  # noqa: E402

F32 = mybir.dt.float32
F32R = mybir.dt.float32r
BF16 = mybir.dt.bfloat16
AX = mybir.AxisListType.X
AF = mybir.ActivationFunctionType

N_CORES = 8
N_NODES = 64          # N
D_MODEL = 256         # D
DK = 128
S_WIN = 10            # window size
N_PAT = 8             # patterns
SL = 512              # tokens per slice
INJ0 = S_WIN * N_NODES  # 640: first injected token


def build_program(Bs: int, T: int) -> bass.Bass:
    TOK = T * N_NODES
    nsl = TOK // SL
    assert TOK % SL == 0
    scale = 1.0 / float(np.sqrt(DK))

    nc = bacc.Bacc("TRN2", target_bir_lowering=False, debug=False)

    x_in = nc.dram_tensor("x", [Bs, T, N_NODES, D_MODEL], F32R, kind="ExternalInput")
    wts = {
        k: nc.dram_tensor(f"wt{k}", [2, 128, DK], F32R, kind="ExternalInput")
        for k in ("q", "k", "v", "u")
    }
    biases_in = {
        k: nc.dram_tensor(f"b{k}", [DK, 1], F32, kind="ExternalInput")
        for k in ("q", "k", "v", "u")
    }
    mT_in = nc.dram_tensor("mT", [DK, S_WIN * N_PAT], F32R, kind="ExternalInput")
    csum_in = nc.dram_tensor("csum", [N_PAT, DK], F32R, kind="ExternalInput")
    idf_in = nc.dram_tensor("idf", [128, 128], F32, kind="ExternalInput")
    idr_in = nc.dram_tensor("idr", [128, 128], F32R, kind="ExternalInput")
    idb_in = nc.dram_tensor("idb", [128, 128], BF16, kind="ExternalInput")
    ones8_in = nc.dram_tensor("ones8", [N_PAT, N_PAT], F32R, kind="ExternalInput")
    sel4_in = nc.dram_tensor("sel4", [128, N_PAT], F32R, kind="ExternalInput")
    out_d = nc.dram_tensor("out", [Bs, T, N_NODES, DK], F32, kind="ExternalOutput")

    x_flat = x_in.rearrange("b t n d -> b (t n) d")
    out_flat = out_d.rearrange("b t n d -> b (t n) d")

    with tile.TileContext(nc) as tc:
        with (
            tc.tile_pool(name="consts", bufs=1) as cpool,
            tc.tile_pool(name="stream", bufs=3) as spool,
            tc.tile_pool(name="ut", bufs=1) as utpool,
            tc.tile_pool(name="attn", bufs=4) as apool,
            tc.tile_pool(name="psA", bufs=2, space="PSUM") as psA,
            tc.tile_pool(name="psK", bufs=1, space="PSUM") as psK,
            tc.tile_pool(name="psS", bufs=2, space="PSUM") as psS,
            tc.tile_pool(name="psT", bufs=3, space="PSUM") as psT,
        ):
            # ---- constants into SBUF ----
            wt_sb = {}
            b_sb = {}
            for k in ("q", "k", "v", "u"):
                wt_sb[k] = cpool.tile([128, 2, DK], F32R, tag=f"wt{k}", name=f"wt{k}_sb")
                nc.sync.dma_start(out=wt_sb[k], in_=wts[k].rearrange("c d m -> d c m"))
                b_sb[k] = cpool.tile([DK, 1], F32, tag=f"b{k}", name=f"b{k}_sb")
                nc.sync.dma_start(out=b_sb[k], in_=biases_in[k][:, :])
            mT_sb = cpool.tile([DK, S_WIN * N_PAT], F32R, tag="mT")
            nc.sync.dma_start(out=mT_sb, in_=mT_in[:, :])
            csum_sb = cpool.tile([N_PAT, DK], F32R, tag="csum")
            nc.sync.dma_start(out=csum_sb, in_=csum_in[:, :])
            idf_sb = cpool.tile([128, 128], F32, tag="idf")
            nc.sync.dma_start(out=idf_sb, in_=idf_in[:, :])
            idb_sb = cpool.tile([128, 128], BF16, tag="idb")
            nc.sync.dma_start(out=idb_sb, in_=idb_in[:, :])
            ones8_sb = cpool.tile([N_PAT, N_PAT], F32R, tag="ones8")
            nc.sync.dma_start(out=ones8_sb, in_=ones8_in[:, :])
            idr_sb = cpool.tile([128, 128], F32R, tag="idr")
            nc.sync.dma_start(out=idr_sb, in_=idr_in[:, :])
            sel4_sb = cpool.tile([128, N_PAT], F32R, tag="sel4")
            nc.sync.dma_start(out=sel4_sb, in_=sel4_in[:, :])

            # Absorb const-DMA semaphores into dedicated PE transposes:
            # walrus's self-loading matmul allows at most 2 sync waits, so
            # real matmuls must never be the first reader of a const DMA.
            def absorb(t, ident, dt):
                p, f = t.shape[0], int(np.prod(t.shape[1:]))
                scr = psA.tile([128, 128], dt, tag="big", name="absorb_scr")
                nc.tensor.transpose(
                    out=scr[0:f, 0:p], in_=t, identity=ident[0:p, 0:p]
                )

            for k in ("q", "k", "v", "u"):
                for cd in range(2):
                    absorb(wt_sb[k][:, cd, :], idr_sb, F32R)
            absorb(mT_sb, idr_sb, F32R)
            absorb(csum_sb, idr_sb, F32R)
            absorb(ones8_sb, idr_sb, F32R)
            absorb(sel4_sb, idr_sb, F32R)
            absorb(idr_sb, idr_sb, F32R)
            absorb(idf_sb, idf_sb, F32)
            absorb(idb_sb, idb_sb, BF16)

            for b in range(Bs):
                ut = utpool.tile([128, TOK], F32R, tag="ut")
                for c in range(nsl):
                    tok0 = c * SL
                    # ---- load x slice [512 tok, 256 d] as [128, 4, 256] ----
                    xn = spool.tile([128, 4, D_MODEL], F32R, tag="xn")
                    nc.sync.dma_start(
                        out=xn,
                        in_=x_flat[b, tok0 : tok0 + SL, :].rearrange(
                            "(j p) d -> p j d", p=128
                        ),
                    )
                    # ---- transpose to xT chunks [128 d, 512 tok] ----
                    xt = []
                    for cd in range(2):
                        tr_ps = psA.tile([128, SL], F32R, tag="big")
                        for j in range(4):
                            nc.tensor.transpose(
                                out=tr_ps[:, j * 128 : (j + 1) * 128],
                                in_=xn[:, j, cd * 128 : (cd + 1) * 128],
                                identity=idr_sb,
                            )
                        xt_c = spool.tile([128, SL], F32R, tag=f"xt{cd}")
                        nc.scalar.copy(out=xt_c, in_=tr_ps)
                        xt.append(xt_c)

                    def linear(key, ps_pool, tag, stop=True):
                        ps = ps_pool.tile([128, SL], F32, tag=tag, name=f"{key}_ps")
                        for cd in range(2):
                            nc.tensor.matmul(
                                ps,
                                lhsT=wt_sb[key][:, cd, :],
                                rhs=xt[cd],
                                start=(cd == 0),
                                stop=(cd == 1) and stop,
                            )
                        return ps

                    # ---- u linear -> UT[,:tok] (fp32, +bias) ----
                    u_ps = linear("u", psA, "big")
                    nc.scalar.activation(
                        out=ut[:, tok0 : tok0 + SL],
                        in_=u_ps,
                        func=AF.Identity,
                        bias=b_sb["u"],
                    )

                    # ---- pattern pipeline ----
                    w_t = None
                    j0 = 0
                    if c >= 1:
                        j0 = 128 if c == 1 else 0
                        nsim = SL - j0
                        sim_ps = psS.tile([N_PAT, SL], F32, tag="small")
                        for s in range(S_WIN):
                            ucol = tok0 + j0 - INJ0 + 64 * s
                            nc.tensor.matmul(
                                sim_ps[:, j0:],
                                lhsT=mT_sb[:, s * N_PAT : (s + 1) * N_PAT],
                                rhs=ut[:, ucol : ucol + nsim],
                                start=(s == 0),
                                stop=(s == S_WIN - 1),
                            )
                        e_t = spool.tile([N_PAT, SL], F32R, tag="e")
                        nc.scalar.activation(
                            out=e_t[:, j0:], in_=sim_ps[:, j0:], func=AF.Exp
                        )
                        den_ps = psS.tile([N_PAT, SL], F32, tag="small")
                        nc.tensor.matmul(
                            den_ps[:, j0:],
                            lhsT=ones8_sb,
                            rhs=e_t[:, j0:],
                            start=True,
                            stop=True,
                        )
                        rd_t = spool.tile([N_PAT, SL], F32, tag="rd")
                        nc.vector.reciprocal(out=rd_t[:, j0:], in_=den_ps[:, j0:])
                        w_t = spool.tile([N_PAT, SL], F32R, tag="w")
                        if j0 > 0:
                            nc.vector.memset(w_t[:, 0:j0].bitcast(F32), 0.0)
                        nc.vector.tensor_mul(
                            out=w_t[:, j0:],
                            in0=e_t[:, j0:],
                            in1=rd_t[:, j0:],
                        )

                    # ---- K linear + r injection ----
                    k_ps = linear("k", psK, "kp", stop=(w_t is None))
                    if w_t is not None:
                        nc.tensor.matmul(
                            k_ps,
                            lhsT=csum_sb,
                            rhs=w_t,
                            start=False,
                            stop=True,
                        )
                    kt_bf = spool.tile([128, SL], BF16, tag="kt")
                    nc.vector.tensor_scalar_add(out=kt_bf, in0=k_ps, scalar1=b_sb["k"])

                    # ---- Q linear ----
                    q_ps = linear("q", psA, "big")
                    qt_bf = spool.tile([128, SL], BF16, tag="qt")
                    nc.vector.tensor_scalar_add(out=qt_bf, in0=q_ps, scalar1=b_sb["q"])

                    # ---- V linear -> bf16 VT -> transpose to V natural ----
                    v_ps = linear("v", psA, "big")
                    vt_bf = spool.tile([128, SL], BF16, tag="vt")
                    nc.scalar.activation(
                        out=vt_bf, in_=v_ps, func=AF.Identity, bias=b_sb["v"]
                    )
                    vnat = spool.tile([64, 8, 128], BF16, tag="vnat")
                    for half in range(2):
                        vtr_ps = psS.tile([64, 4, 128], BF16, tag="small")
                        for j in range(4):
                            tj = half * 4 + j
                            nc.tensor.transpose(
                                out=vtr_ps[:, j, :],
                                in_=vt_bf[:, tj * 64 : (tj + 1) * 64],
                                identity=idb_sb,
                            )
                        nc.scalar.copy(
                            out=vnat[:, half * 4 : (half + 1) * 4, :], in_=vtr_ps
                        )

                    # ---- attention: 4 pairs of timesteps ----
                    out_sb = spool.tile([128, 4, DK], F32, tag="osb")
                    for pr in range(4):
                        c1 = pr * 128
                        c2 = pr * 128 + 64
                        sc_ps = psT.tile([128, 64], F32, tag="at")
                        nc.tensor.matmul(
                            sc_ps[0:64, :],
                            lhsT=qt_bf[:, c1 : c1 + 64],
                            rhs=kt_bf[:, c1 : c1 + 64],
                            start=True,
                            stop=True,
                        )
                        nc.tensor.matmul(
                            sc_ps[64:128, :],
                            lhsT=qt_bf[:, c2 : c2 + 64],
                            rhs=kt_bf[:, c2 : c2 + 64],
                            start=True,
                            stop=True,
                            tile_position=(0, 64),
                        )
                        mx = apool.tile([128, 1], F32, tag="mx")
                        nc.vector.reduce_max(out=mx, in_=sc_ps, axis=AX, negate=True)
                        mxs = apool.tile([128, 1], F32, tag="mxs")
                        nc.vector.tensor_scalar_mul(out=mxs, in0=mx, scalar1=scale)
                        attn_bf = apool.tile([128, 64], BF16, tag="attn")
                        ssum = apool.tile([128, 1], F32, tag="ss")
                        nc.scalar.activation(
                            out=attn_bf,
                            in_=sc_ps,
                            func=AF.Exp,
                            bias=mxs,
                            scale=scale,
                            accum_out=ssum,
                        )
                        rs = apool.tile([128, 1], F32, tag="rs")
                        nc.vector.reciprocal(out=rs, in_=ssum)
                        at_ps = psT.tile([64, 128], BF16, tag="at")
                        nc.tensor.transpose(out=at_ps, in_=attn_bf, identity=idb_sb)
                        at_bf = apool.tile([64, 128], BF16, tag="atbf")
                        nc.vector.tensor_copy(out=at_bf, in_=at_ps)
                        o_ps = psT.tile([128, 128], F32, tag="at")
                        nc.tensor.matmul(
                            o_ps[0:64, :],
                            lhsT=at_bf[:, 0:64],
                            rhs=vnat[:, 2 * pr, :],
                            start=True,
                            stop=True,
                        )
                        nc.tensor.matmul(
                            o_ps[64:128, :],
                            lhsT=at_bf[:, 64:128],
                            rhs=vnat[:, 2 * pr + 1, :],
                            start=True,
                            stop=True,
                            tile_position=(0, 64),
                        )
                        nc.vector.tensor_scalar_mul(
                            out=out_sb[:, pr, :], in0=o_ps, scalar1=rs
                        )

                    nc.sync.dma_start(
                        out=out_flat[b, tok0 : tok0 + SL, :].rearrange(
                            "(j p) d -> p j d", p=128
                        ),
                        in_=out_sb,
                    )
    nc.finalize()
    return nc


def _host_prep(inputs: dict) -> dict:
    f = np.float32
    aux = {}
    for k, (W, bias) in {
        "q": (inputs["WQ"], inputs["bQ"]),
        "k": (inputs["WK"], inputs["bK"]),
        "v": (inputs["WV"], inputs["bV"]),
        "u": (inputs["Wu"], inputs["bu"]),
    }.items():
        aux[f"wt{k}"] = np.ascontiguousarray(
            np.asarray(W, f).T.reshape(2, 128, DK)
        )
        aux[f"b{k}"] = np.ascontiguousarray(np.asarray(bias, f).reshape(DK, 1))
    patterns = np.asarray(inputs["patterns"], f)
    m = patterns @ np.asarray(inputs["Wm"], f).T + np.asarray(inputs["bm"], f)
    aux["mT"] = np.ascontiguousarray(m.transpose(2, 1, 0).reshape(DK, S_WIN * N_PAT))
    aux["csum"] = np.ascontiguousarray(
        (patterns @ np.asarray(inputs["Wc"], f).T + np.asarray(inputs["bc"], f)).sum(
            axis=1
        )
    )
    aux["idf"] = np.eye(128, dtype=f)
    aux["idr"] = np.eye(128, dtype=f)
    aux["idb"] = np.eye(128, dtype=ml_dtypes.bfloat16)
    aux["ones8"] = np.ones([N_PAT, N_PAT], f)
    sel4 = np.zeros([128, N_PAT], f)
    for g in range(4):
        for p in range(N_PAT):
            sel4[32 * g + p, p] = 1.0
    aux["sel4"] = sel4
    return aux


TRACE = False
LAST_RESULTS = None


def kernel(**inputs) -> np.ndarray:
    global LAST_RESULTS
    from concourse.bass_utils import run_bass_kernel_spmd

    x = np.asarray(inputs["x"], np.float32)
    B, T = x.shape[0], x.shape[1]
    bs = B // N_CORES
    aux = _host_prep(inputs)
    nc = build_program(bs, T)
    in_maps = [dict(aux, x=x[i * bs : (i + 1) * bs]) for i in range(N_CORES)]
    res = run_bass_kernel_spmd(nc, in_maps, list(range(N_CORES)), trace=TRACE)
    LAST_RESULTS = res
    return np.concatenate([r["out"] for r in res.results], axis=0)

